# revision 35
# baseline (speedup 1.0000x reference)
"""AdaptiveGeometryAttention distributed Bass kernel for 8 trn2 NeuronCores.

Sharding: data-parallel over B (2 groups of 4 cores), head-parallel over H
(4 heads per core). Each core computes its heads' attention and a partial
out-projection [T, C]; per-row-block ReduceScatter(add) chunks over each
4-core group overlap the tail collective with compute.

Design notes (v3):
- bf16 x/weights on host: halves load traffic; PE runs bf16 at 1 cyc/row.
  An extra bf16 residual of x feeds the importance/alpha projection so the
  hard spike threshold (imp > 0.5) cannot flip on rounding.
- Stats phase packs all 8 (side, head) norm rows into partitions 0-7 of one
  tile so each sinh/cosh/rat op covers everything in one 1024-col pass.
- Per-unit attention chain is 2 Act passes (Ln straight from the ni PSUM -
  the clamp is dropped since min ni = 207 over all pairs; Exp with den
  accum) + 3 DVE passes (d2a = L*L; Bt = se_psum - alpha*d2a via STT
  reading PSUM; 128-col diag mask add). Softmax normalization (spike/den)
  is applied to the tiny [d,t] attnv outputs via a PE-broadcast of sc2
  instead of an S-wide pass over the probabilities.
- Tile emits semaphore-recycle fences that wait on the collective clock at
  their program point, so ALL ReduceScatters are EMITTED after the compute
  pipeline while the gpsimd queue is kept empty during phase D: the queue
  head reaches them immediately and each RS still executes early, paced by
  its partial-store DMA semaphore, but no mid-pipeline fence ever waits on
  a collective. RS runs in 4 chunks of [256, C] to amortize the ~12us
  per-collective rendezvous latency; tail loads/stores ride gpsimd.
"""
import os
import sys

for _p in ("/opt/trn_rl_repo",):
    if _p not in sys.path:
        sys.path.append(_p)

import ml_dtypes
import numpy as np
import concourse.bass as bass
import concourse.bacc as bacc
import concourse.mybir as mybir
from concourse import masks
from concourse.alu_op_type import AluOpType
from concourse.tile import TileContext
from concourse.bass_utils import run_bass_kernel_spmd

AF = mybir.ActivationFunctionType
DT = mybir.dt

B, T, C, H, D = 2, 1024, 1024, 16, 64
HL = 4                 # heads per core
JD = HL * D            # 256 local head dims
N_CORES = 8
GROUPS = [[0, 1, 2, 3], [4, 5, 6, 7]]
SQD = 0.125            # 1/sqrt(D)
NEG = -1.0e30
BF16 = ml_dtypes.bfloat16

KSTATS = {}

# The act-table-load placement pass picks the FIRST set containing each
# activation function; strip ln/exp from the single-function sets so the
# combined natural_log_exp_and_others set is chosen (avoids table thrash).
_orig_get_tables = bacc.get_activation_tables


def _patched_get_tables(arch):
    t = _orig_get_tables(arch)
    for nm in ("exp_and_others", "natural_log", "exp_and_friends"):
        if nm in t:
            t[nm] = t[nm] - {AF.Exp, AF.Ln}
    return t


bacc.get_activation_tables = _patched_get_tables


def _f32r(ap):
    return ap.bitcast(DT.float32r)


def build_nc():
    nc = bacc.Bacc("TRN2")

    # ---- I/O ----
    xT_e = nc.dram_tensor("xT", [C, T], DT.bfloat16, kind="ExternalInput")
    xlT_e = nc.dram_tensor("xlT", [C, T], DT.bfloat16, kind="ExternalInput")
    wqT_e = nc.dram_tensor("wqT", [C, JD], DT.bfloat16, kind="ExternalInput")
    wkT_e = nc.dram_tensor("wkT", [C, JD], DT.bfloat16, kind="ExternalInput")
    wvT_e = nc.dram_tensor("wvT", [C, JD], DT.bfloat16, kind="ExternalInput")
    bqd_e = nc.dram_tensor("bqd", [128, 2], DT.float32, kind="ExternalInput")
    bkd_e = nc.dram_tensor("bkd", [128, 2], DT.float32, kind="ExternalInput")
    bvd_e = nc.dram_tensor("bvd", [128, 2], DT.float32, kind="ExternalInput")
    wia_e = nc.dram_tensor("wia", [C, 5], DT.bfloat16, kind="ExternalInput")
    biad_e = nc.dram_tensor("biad", [128, 1], DT.float32, kind="ExternalInput")
    woT_e = nc.dram_tensor("woT", [JD, C], DT.bfloat16, kind="ExternalInput")
    bout_e = nc.dram_tensor("bout_b", [96, C], DT.float32, kind="ExternalInput")
    th_e = nc.dram_tensor("th_b", [128, 1], DT.float32, kind="ExternalInput")
    cmask_e = nc.dram_tensor("cmask", [128, 128], DT.float32, kind="ExternalInput")
    selD_e = nc.dram_tensor("selD", [128, 32], DT.float32r, kind="ExternalInput")
    selB_e = nc.dram_tensor("selB", [128, 512], DT.float32r, kind="ExternalInput")
    selO_e = nc.dram_tensor("selO", [128, 256], DT.float32r, kind="ExternalInput")
    selY_e = nc.dram_tensor("selY", [4, 256], DT.float32r, kind="ExternalInput")
    out_e = nc.dram_tensor("out", [T // 4, C], DT.float32, kind="ExternalOutput")

    # one tensor per row-block chunk: avoids false WAR deps between a
    # chunk's ReduceScatter read and the next chunk's partial store
    # group sizes chosen to spread the RS doorbells evenly and make the
    # final (tail-critical) chunk the smallest
    GSZ = [384, 256, 256, 128]
    partial_d = [nc.dram_tensor(f"partial_d{i}", [GSZ[i], C], DT.bfloat16)
                 for i in range(4)]
    rs_out_d = [nc.dram_tensor(f"rs_out_d{i}", [GSZ[i] // 4, C], DT.bfloat16)
                for i in range(4)]
    TI2G = {0: (0, 0), 1: (0, 128), 2: (0, 256), 3: (1, 0),
            4: (1, 128), 5: (2, 0), 6: (2, 128), 7: (3, 0)}

    with TileContext(nc) as tc:
        with (
            tc.tile_pool(name="const", bufs=1) as cpool,
            tc.tile_pool(name="mainp", bufs=1) as mp,
        ):
            # ---- constants (small, on the gpsimd/vector queues) ----
            idf = cpool.tile([128, 128], DT.float32, tag="idf")
            masks.make_identity(nc, idf[:])
            cmask = cpool.tile([128, 128], DT.float32, tag="cmask")
            nc.gpsimd.dma_start(out=cmask[:], in_=cmask_e[:])
            selD = cpool.tile([128, 32], DT.float32r, tag="selD")
            nc.gpsimd.dma_start(out=selD[:], in_=selD_e[:])
            selB = cpool.tile([128, 512], DT.float32r, tag="selB")
            nc.gpsimd.dma_start(out=selB[:], in_=selB_e[:])
            selO = cpool.tile([128, 256], DT.float32r, tag="selO")
            nc.gpsimd.dma_start(out=selO[:], in_=selO_e[:])
            selY = cpool.tile([4, 256], DT.float32r, tag="selY")
            nc.gpsimd.dma_start(out=selY[:], in_=selY_e[:])
            th_b = cpool.tile([128, 1], DT.float32, tag="thb")
            nc.gpsimd.dma_start(out=th_b[:], in_=th_e[:])
            biad = cpool.tile([128, 1], DT.float32, tag="biad")
            nc.gpsimd.dma_start(out=biad[:], in_=biad_e[:])
            bout_b = cpool.tile([96, C], DT.float32, tag="boutb")
            nc.gpsimd.dma_start(out=bout_b[:], in_=bout_e[:])

            # ---- persistent main tiles ----
            qb2T = [mp.tile([128, T], DT.bfloat16, tag=f"qb2T{j}", name=f"qb2T{j}") for j in range(2)]
            kbT = [mp.tile([128, T], DT.bfloat16, tag=f"kbT{j}", name=f"kbT{j}") for j in range(2)]
            qhT = [mp.tile([128, T], DT.float32r, tag=f"qhT{j}", name=f"qhT{j}") for j in range(2)]
            khT = [mp.tile([128, T], DT.float32r, tag=f"khT{j}", name=f"khT{j}") for j in range(2)]
            vbf = mp.tile([128, 8 * JD], DT.bfloat16, tag="vbf")
            wobf = mp.tile([128, 2 * C], DT.bfloat16, tag="wobf")
            for cc in range(2):
                nc.gpsimd.dma_start(
                    out=wobf[:, cc * C:(cc + 1) * C],
                    in_=woT_e[cc * 128:(cc + 1) * 128, :],
                )
            sat = mp.tile([128, 64], DT.float32, tag="sat")
            nsat = mp.tile([128, 64], DT.float32, tag="nsat")
            spike = mp.tile([128, 8], DT.float32, tag="spike")

            with tc.tile_pool(name="wB", bufs=1) as wb, \
                 tc.tile_pool(name="psB", bufs=3, space="PSUM") as psB:
                # ---- load x/weights; wv first so the v projection can
                # start as soon as the matching xT chunks land ----
                # even wv/x chunks on sync, everything else on the scalar
                # queue: Sync's phase-D job is ONLY the pT transposes, so its
                # semaphore epochs never wait on slow consumers; Act does no
                # DMAs after the load phase for the same reason.
                xT = wb.tile([128, 8 * T], DT.bfloat16, tag="xT")
                wv = wb.tile([128, 8 * JD], DT.bfloat16, tag="wv")
                wq = wb.tile([128, 8 * JD], DT.bfloat16, tag="wq")
                wk = wb.tile([128, 8 * JD], DT.bfloat16, tag="wk")
                wiar = wb.tile([128, 40], DT.bfloat16, tag="wiar")
                xlT = wb.tile([128, 8 * T], DT.bfloat16, tag="xlT")
                for kc in range(8):
                    nc.sync.dma_start(
                        out=wq[:, kc * JD:(kc + 1) * JD],
                        in_=wqT_e[kc * 128:(kc + 1) * 128, :],
                    )
                    nc.scalar.dma_start(
                        out=wk[:, kc * JD:(kc + 1) * JD],
                        in_=wkT_e[kc * 128:(kc + 1) * 128, :],
                    )
                for kc in range(8):
                    q = nc.sync if kc % 2 == 0 else nc.scalar
                    q.dma_start(
                        out=xT[:, kc * T:(kc + 1) * T],
                        in_=xT_e[kc * 128:(kc + 1) * 128, :],
                    )
                for kc in range(8):
                    q = nc.sync if kc % 2 == 0 else nc.scalar
                    q.dma_start(
                        out=wv[:, kc * JD:(kc + 1) * JD],
                        in_=wvT_e[kc * 128:(kc + 1) * 128, :],
                    )
                # xl: bf16 residual of x, for the threshold-sensitive
                # importance/alpha projection (imp > th is a hard compare)
                for w_t, w_e, jw, q in ((wiar, wia_e, 5, nc.scalar),
                                        (xlT, xlT_e, T, nc.gpsimd)):
                    for kc in range(8):
                        q.dma_start(
                            out=w_t[:, kc * jw:(kc + 1) * jw],
                            in_=w_e[kc * 128:(kc + 1) * 128, :],
                        )
                bqd = wb.tile([128, 2], DT.float32, tag="bqd")
                bkd = wb.tile([128, 2], DT.float32, tag="bkd")
                bvd = wb.tile([128, 2], DT.float32, tag="bvd")
                nc.scalar.dma_start(out=bqd[:], in_=bqd_e[:])
                nc.scalar.dma_start(out=bkd[:], in_=bkd_e[:])
                nc.scalar.dma_start(out=bvd[:], in_=bvd_e[:])

                # ---- phase B: projections, weight-stationary [d, t] ----
                qT = [wb.tile([128, T], DT.float32, tag=f"qT{j}", name=f"qT{j}")
                      for j in range(2)]
                kT = [wb.tile([128, T], DT.float32, tag=f"kT{j}", name=f"kT{j}")
                      for j in range(2)]
                for dsts, w_t, b_t in ((qT, wq, bqd), (kT, wk, bkd)):
                    for jc in range(2):
                        pw = psB.tile([128, 1024], DT.float32, tag="ps")
                        for tc2 in range(2):
                            for kc in range(8):
                                nc.tensor.matmul(
                                    pw[:, tc2 * 512:(tc2 + 1) * 512],
                                    w_t[:, kc * JD + jc * 128:
                                        kc * JD + (jc + 1) * 128],
                                    xT[:, kc * T + tc2 * 512:
                                       kc * T + tc2 * 512 + 512],
                                    start=(kc == 0), stop=(kc == 7),
                                )
                        nc.vector.tensor_scalar(
                            dsts[jc][:], pw[:], b_t[:, jc:jc + 1], None,
                            AluOpType.add
                        )

                for jc in range(2):
                    pw = psB.tile([128, 1024], DT.float32, tag="ps")
                    for tc2 in range(2):
                        for kc in range(8):
                            nc.tensor.matmul(
                                pw[:, tc2 * 512:(tc2 + 1) * 512],
                                wv[:, kc * JD + jc * 128:
                                   kc * JD + (jc + 1) * 128],
                                xT[:, kc * T + tc2 * 512:
                                   kc * T + tc2 * 512 + 512],
                                start=(kc == 0), stop=(kc == 7),
                            )
                    vtmp = wb.tile([128, T], DT.bfloat16, tag=f"vtmp{jc}",
                                   name=f"vtmp{jc}")
                    nc.vector.tensor_scalar(
                        vtmp[:], pw[:], bvd[:, jc:jc + 1], None, AluOpType.add
                    )
                    # [d,t] -> row layout [s, d] per 128-block
                    nc.scalar.dma_start_transpose(
                        vbf[:].rearrange("p (s c) -> p s c", c=JD)
                        [:, :, jc * 128:(jc + 1) * 128],
                        vtmp[:],
                    )

                # ia projection: rows 0-4 = imp, alpha h0-3
                iaS = wb.tile([128, T], DT.float32, tag="iaS")
                nc.gpsimd.memset(iaS[:], 0.0)
                pia = psB.tile([128, 1024], DT.float32, tag="ps", name="pia")
                for tc2 in range(2):
                    for kc in range(16):
                        xsrc = xT if kc < 8 else xlT
                        kc8 = kc % 8
                        nc.tensor.matmul(
                            pia[0:5, tc2 * 512:(tc2 + 1) * 512],
                            wiar[:, kc8 * 5:(kc8 + 1) * 5],
                            xsrc[:, kc8 * T + tc2 * 512:
                                 kc8 * T + tc2 * 512 + 512],
                            start=(kc == 0), stop=(kc == 15),
                        )
                nc.scalar.activation(iaS[0:5, :], pia[0:5, :], AF.Sigmoid,
                                     bias=biad[0:5, :])
                # importance[t=0] := 0 before threshold compare
                nc.vector.memset(iaS[0:1, 0:1], 0.0)
                # oma rows 32-36: 1 - (imp, alpha) (row 32 unused)
                nc.vector.tensor_scalar(
                    iaS[32:37, :], iaS[0:5, :], -1.0, 1.0,
                    AluOpType.mult, AluOpType.add,
                )

                # ---- phase C: per-(side, head) norm stats in rows 0-7 ----
                # S8/sq/omar are f32r: the BIR verifier requires f32r matmul
                # operands to be emitted as f32r by their producers (DVE ops
                # round on write; memset goes through an f32 view).
                S8 = wb.tile([128, T], DT.float32r, tag="S8")
                nc.gpsimd.memset(S8[:].bitcast(DT.float32), 0.0)
                omar = wb.tile([128, T], DT.float32r, tag="omar")
                nc.vector.tensor_copy(omar[0:8, :], iaS[32:40, :])
                t1 = wb.tile([128, T], DT.float32, tag="t1")
                n8 = wb.tile([128, T], DT.float32, tag="n8")
                ep = wb.tile([128, T], DT.float32, tag="ep")
                em = wb.tile([128, T], DT.float32, tag="em")
                sq = [wb.tile([128, T], DT.float32r, tag=f"sq{i}",
                              name=f"sq{i}") for i in range(2)]
                pnsq = psB.tile([128, 1024], DT.float32, tag="ps", name="pnsq")
                srcs4 = [qT[0], qT[1], kT[0], kT[1]]
                for i, src in enumerate(srcs4):
                    sqt = sq[i % 2]
                    nc.vector.tensor_mul(sqt[:], src[:], src[:])
                    for tc2 in range(2):
                        nc.tensor.matmul(
                            pnsq[0:8, tc2 * 512:(tc2 + 1) * 512],
                            selD[:, i * 8:(i + 1) * 8],
                            sqt[:, tc2 * 512:(tc2 + 1) * 512],
                            start=(i == 0), stop=(i == 3),
                        )
                # n = exp(0.5 ln n^2); sinh/cosh from exp(+-n); 1/n as
                # exp(-0.5 ln n^2) (DVE reciprocal costs ~6.5us here)
                nc.scalar.activation(t1[0:8, :], pnsq[0:8, :], AF.Ln)
                nc.scalar.activation(n8[0:8, :], t1[0:8, :], AF.Exp, scale=0.5)
                nc.scalar.activation(ep[0:8, :], n8[0:8, :], AF.Exp)
                nc.scalar.activation(em[0:8, :], n8[0:8, :], AF.Exp, scale=-1.0)
                nr8 = wb.tile([128, T], DT.float32, tag="nr8")
                nc.scalar.activation(nr8[0:8, :], t1[0:8, :], AF.Exp,
                                     scale=-0.5)
                nc.vector.tensor_add(S8[32:40, :], ep[0:8, :], em[0:8, :])
                nc.vector.tensor_sub(t1[0:8, :], ep[0:8, :], em[0:8, :])
                nc.vector.tensor_mul(S8[0:8, :], t1[0:8, :], nr8[0:8, :])

                # ---- phase C2: modified q/k via broadcast matmuls ----
                # qb2T needs the ORIGINAL q rows; kbT is a cast of kT; then
                # overwrite q/k d0 rows with 1.0 so the rat/cosh broadcast
                # lands cosh directly in qhT/khT rows 0 and 64.
                for jc in range(2):
                    pbo = psB.tile([128, 1024], DT.float32, tag="ps",
                                   name=f"pbo{jc}")
                    for tc2 in range(2):
                        sl = slice(tc2 * 512, (tc2 + 1) * 512)
                        nc.tensor.matmul(
                            pbo[:, sl], selO[0:8, jc * 128:(jc + 1) * 128],
                            omar[0:8, sl],
                            start=True, stop=True,
                        )
                    nc.vector.tensor_mul(qb2T[jc][:], qT[jc][:], pbo[:])
                    nc.scalar.copy(kbT[jc][:], kT[jc][:])
                for jc in range(2):
                    for rt in (qT[jc], kT[jc]):
                        nc.gpsimd.memset(rt[0:1, :], 1.0)
                        nc.gpsimd.memset(rt[64:65, :], 1.0)
                for side, (srcs, dsts) in enumerate(((qT, qhT), (kT, khT))):
                    for jc in range(2):
                        pbb = psB.tile([128, 1024], DT.float32, tag="ps",
                                       name=f"pbb{side}{jc}")
                        for tc2 in range(2):
                            sl = slice(tc2 * 512, (tc2 + 1) * 512)
                            nc.tensor.matmul(
                                pbb[:, sl],
                                selB[0:48, (side * 2 + jc) * 128:
                                     (side * 2 + jc + 1) * 128],
                                S8[0:48, sl],
                                start=True, stop=True,
                            )
                        nc.vector.tensor_mul(dsts[jc][:], srcs[jc][:], pbb[:])

                # transposed per-t stats: sat cols t8*8 + (imp, alpha h0-3)
                pst = psB.tile([128, 1024], DT.float32, tag="ps", name="pst")
                for t8 in range(8):
                    nc.tensor.transpose(
                        pst[:, t8 * 128:(t8 + 1) * 128],
                        iaS[:, t8 * 128:(t8 + 1) * 128],
                        idf[:],
                    )
                nc.vector.tensor_copy(
                    sat[:].rearrange("p (b c) -> p b c", c=8)[:, :, 0:5],
                    pst[:].rearrange("p (b c) -> p b c", c=128)[:, :, 0:5],
                )
                nc.vector.tensor_scalar(nsat[:], sat[:], -1.0, None,
                                        AluOpType.mult)
                nc.vector.tensor_scalar(spike[:], sat[:, 0::8], th_b[:], None,
                                        AluOpType.is_gt)

            # ---- phase D: attention, software-pipelined ----
            # per unit (ti, h): PE se/ni; Act Ln(ni psum); DVE d2a=L*L,
            # Bt=se-alpha*d2a (STT from psum); Pool diag mask; Act Exp+accum.
            # per ti: DMA transposes -> pT; PE attnv + sc2 broadcast; DVE yT
            # scale; PE out-proj; Pool out copy; Act partial store; RS.
            with tc.tile_pool(name="pipe", bufs=1) as pp, \
                 tc.tile_pool(name="pipes", bufs=4) as sp, \
                 tc.tile_pool(name="pipeo", bufs=4) as po, \
                 tc.tile_pool(name="psS", bufs=2, space="PSUM") as psS, \
                 tc.tile_pool(name="psN", bufs=2, space="PSUM") as psN:
                UNITS = [(ti, h) for ti in range(8) for h in range(HL)]
                NU = len(UNITS)
                st = {}
                den_t = {}
                pT_t = {}
                sc_t = {}

                def stage0(u):
                    ti, h = UNITS[u]
                    S = (ti + 1) * 128
                    jc, hh = h // 2, h % 2
                    se = psS.tile([128, 1024], DT.float32, tag="se", name=f"se{u}")
                    ni = psN.tile([128, 1024], DT.float32, tag="ni", name=f"ni{u}")
                    for c0 in range(0, S, 512):
                        n_sc = min(512, S - c0)
                        nc.tensor.matmul(
                            se[:, c0:c0 + n_sc],
                            qb2T[jc][hh * 64:(hh + 1) * 64, ti * 128:(ti + 1) * 128],
                            kbT[jc][hh * 64:(hh + 1) * 64, c0:c0 + n_sc],
                            start=True, stop=True,
                        )
                    for c0 in range(0, S, 512):
                        n_sc = min(512, S - c0)
                        nc.tensor.matmul(
                            ni[:, c0:c0 + n_sc],
                            qhT[jc][hh * 64:(hh + 1) * 64,
                                    ti * 128:(ti + 1) * 128],
                            khT[jc][hh * 64:(hh + 1) * 64, c0:c0 + n_sc],
                            start=True, stop=True,
                        )
                    st[u] = (se, ni)

                def stage12(u):
                    ti, h = UNITS[u]
                    S = (ti + 1) * 128
                    se, ni = st.pop(u)
                    L = pp.tile([128, 1024], DT.float32, tag="L", bufs=3,
                                name=f"L{u}")
                    nc.scalar.activation(L[:, :S], ni[:, :S], AF.Ln, scale=2.0)
                    d2a = pp.tile([128, 1024], DT.float32, tag="d2a", bufs=3,
                                  name=f"d2a{u}")
                    nc.vector.tensor_mul(d2a[:, :S], L[:, :S], L[:, :S])
                    Bt = pp.tile([128, 1024], DT.float32, tag="Bt", bufs=4,
                                 name=f"Bt{u}")
                    sac = ti * 8 + 1 + h
                    nc.vector.scalar_tensor_tensor(
                        Bt[:, :S], d2a[:, :S], nsat[:, sac:sac + 1], se[:, :S],
                        AluOpType.mult, AluOpType.add,
                    )
                    nc.vector.tensor_add(
                        Bt[:, ti * 128:S], Bt[:, ti * 128:S], cmask[:]
                    )
                    st[u] = Bt

                pb4_t = {}
                psy_t = {}

                def stage3(u):
                    ti, h = UNITS[u]
                    S = (ti + 1) * 128
                    Bt = st.pop(u)
                    if h == 0:
                        den_t[ti] = sp.tile([128, 4], DT.float32, tag="den",
                                            name=f"den{ti}")
                        pb4_t[ti] = pp.tile([128, 4 * 1024], DT.bfloat16,
                                            tag="pb4", bufs=2, name=f"pb4{ti}")
                    den = den_t[ti]
                    pb4 = pb4_t[ti]
                    nc.scalar.activation(pb4[:, h * S:(h + 1) * S],
                                         Bt[:, :S],
                                         AF.Exp,
                                         scale=SQD, accum_out=den[:, h:h + 1])
                    if ti == 7:
                        # last ti: per-head transpose + attnv right after each
                        # Exp so the drain chain is one head, not four. psy7
                        # lives in the se ring (nothing allocs there later).
                        if h == 0:
                            psy_t[ti] = psS.tile([128, 1024], DT.float32,
                                                 tag="se", name="psy7")
                            pT_t[ti] = pp.tile([128, 4 * 1024], DT.bfloat16,
                                               tag="pT4", bufs=3, name="pT47")
                        pT47 = pT_t[ti]
                        nc.sync.dma_start_transpose(
                            pT47[:, h * S:(h + 1) * S].rearrange(
                                "p (b c) -> p b c", c=128),
                            pb4[:, h * S:(h + 1) * S],
                        )
                        jc2, hh2 = h // 2, h % 2
                        for sj in range(ti + 1):
                            nc.tensor.matmul(
                                psy_t[ti][hh2 * 64:(hh2 + 1) * 64,
                                          jc2 * 128:(jc2 + 1) * 128],
                                vbf[:, sj * JD + h * D: sj * JD + (h + 1) * D],
                                pT47[:, h * S + sj * 128:
                                     h * S + (sj + 1) * 128],
                                start=(sj == 0), stop=(sj == ti),
                                tile_position=(0, hh2 * 64),
                            )
                    if h == HL - 1:
                        den4 = den_t.pop(ti)
                        rec4 = sp.tile([128, 4], DT.float32, tag="rec4")
                        nc.vector.reciprocal(rec4[:], den4[:])
                        sc24 = sp.tile([128, 4], DT.float32, tag="sc24",
                                       name=f"sc24{ti}")
                        nc.vector.tensor_scalar(
                            sc24[:], rec4[:], spike[:, ti:ti + 1], None,
                            AluOpType.mult,
                        )
                        sc_t[ti] = sc24
                        pb4 = pb4_t.pop(ti)
                        if ti != 7:
                            pT4 = pp.tile([128, 4 * 1024], DT.bfloat16,
                                          tag="pT4", bufs=3, name=f"pT4{ti}")
                            # heads are packed S-contiguously, so one DMA
                            # transposes exactly the needed 4*S columns
                            nc.sync.dma_start_transpose(
                                pT4[:, :4 * S].rearrange("p (b c) -> p b c",
                                                         c=128),
                                pb4[:, :4 * S],
                            )
                            pT_t[ti] = pT4

                def stage4(u):
                    ti, h = UNITS[u]
                    if h != HL - 1:
                        return
                    pT4 = pT_t.pop(ti)
                    S4 = (ti + 1) * 128
                    sc24 = sc_t.pop(ti)
                    if ti == 7:
                        psy = psy_t.pop(ti)
                    else:
                        psy = psN.tile([128, 1024], DT.float32, tag="ni",
                                       name=f"psy{ti}")
                    # sc2 [t,h] -> [h,t] -> per-head-pair broadcast [d, t]
                    nc.tensor.transpose(psy[0:4, 512:640], sc24[:], idf[:])
                    scT = sp.tile([4, 128], DT.float32r, tag="scT")
                    nc.vector.tensor_copy(scT[:], psy[0:4, 512:640])
                    for half in range(2):
                        nc.tensor.matmul(
                            psy[:, 768 + half * 128:768 + (half + 1) * 128],
                            selY[:, half * 128:(half + 1) * 128],
                            scT[:],
                            start=True, stop=True,
                        )
                    for h2 in (range(HL) if ti != 7 else ()):
                        jc2, hh2 = h2 // 2, h2 % 2
                        for sj in range(ti + 1):
                            nc.tensor.matmul(
                                psy[hh2 * 64:(hh2 + 1) * 64,
                                    jc2 * 128:(jc2 + 1) * 128],
                                vbf[:, sj * JD + h2 * D: sj * JD + (h2 + 1) * D],
                                pT4[:, h2 * S4 + sj * 128:
                                    h2 * S4 + (sj + 1) * 128],
                                start=(sj == 0), stop=(sj == ti),
                                tile_position=(0, hh2 * 64),
                            )
                    sc2b = sp.tile([128, 256], DT.float32, tag="sc2b")
                    nc.vector.tensor_copy(sc2b[:], psy[:, 768:1024])
                    yTm = sp.tile([128, 256], DT.bfloat16, tag="yTm")
                    nc.vector.tensor_mul(yTm[:], psy[:, 0:256], sc2b[:])
                    yT0, yT1 = yTm[:, 0:128], yTm[:, 128:256]
                    pso = psN.tile([128, 1024], DT.float32, tag="ni",
                                   name=f"pso{ti}")
                    for oc in range(2):
                        for cc, yT_t in ((0, yT0), (1, yT1)):
                            nc.tensor.matmul(
                                pso[:, oc * 512:(oc + 1) * 512],
                                yT_t,
                                wobf[:, cc * C + oc * 512: cc * C + oc * 512 + 512],
                                start=(cc == 0), stop=(cc == 1),
                            )
                    out_sb = po.tile([128, 1024], DT.bfloat16, tag="outsb")
                    nc.vector.tensor_copy(out_sb[:], pso[:])
                    # partial store on the Pool queue (Sync carries only the
                    # pT transposes, Act only compute). The RS for this chunk
                    # is issued 2 tis LATER: semaphore-recycle fences wait on
                    # the CC clock at their program point, so late issuance
                    # keeps mid-pipeline fences free of collective waits.
                    g_, off_ = TI2G[ti]
                    nc.sync.dma_start(
                        out=partial_d[g_][off_:off_ + 128, :],
                        in_=out_sb[:])

                def _issue_rs(g):
                    nc.gpsimd.collective_compute(
                        "ReduceScatter", mybir.AluOpType.add,
                        replica_groups=GROUPS,
                        ins=[partial_d[g][:]],
                        outs=[rs_out_d[g][:]],
                    )

                for step in range(NU + 6):
                    if step < NU:
                        stage0(step)
                    if 1 <= step < NU + 1:
                        stage12(step - 1)
                    if 2 <= step < NU + 2:
                        stage3(step - 2)
                    if 6 <= step < NU + 6:
                        stage4(step - 6)

                for g in range(4):
                    _issue_rs(g)

                # ---- tail: bias add + output store per RS chunk, on the
                # Pool queue so Sync never waits on collective sems ----
                ob = 0
                for g in range(4):
                    sz4 = GSZ[g] // 4
                    finb = po.tile([96, 1024], DT.bfloat16, tag="finb")
                    nc.gpsimd.dma_start(out=finb[0:sz4, :], in_=rs_out_d[g][:])
                    fin = po.tile([96, 1024], DT.float32, tag="fin")
                    nc.vector.tensor_add(fin[0:sz4, :], finb[0:sz4, :],
                                         bout_b[0:sz4, :])
                    nc.gpsimd.dma_start(
                        out=out_e[ob:ob + sz4, :], in_=fin[0:sz4, :]
                    )
                    ob += sz4
    nc.finalize()
    return nc


_NC = None


def _get_nc():
    global _NC
    if _NC is None:
        _NC = build_nc()
    return _NC


def _build_sels():
    # selD [128, 4*8]: per (side, jc) block, col 2h+par sums the d rows of
    # head parity par (excluding d0) for the norm^2 reduction
    selD = np.zeros((128, 32), np.float32)
    for i in range(4):            # (q,0),(q,1),(k,0),(k,1)
        side, jc = i // 2, i % 2
        r0 = side * 4 + jc * 2
        selD[1:64, i * 8 + r0] = 1.0
        selD[65:128, i * 8 + r0 + 1] = 1.0
    # selB [128, 4*128]: rat/cosh broadcast. S8 rows: rat at side*4+h,
    # 2cosh at 32+side*4+h. q side +0.5 rat, k side -0.5 (metric sign).
    selB = np.zeros((128, 512), np.float32)
    for i in range(4):
        side, jc = i // 2, i % 2
        rsign = 0.5 if side == 0 else -0.5
        for par in range(2):
            hrow = side * 4 + jc * 2 + par
            selB[32 + hrow, i * 128 + 64 * par] = 0.5
            selB[hrow, i * 128 + 64 * par + 1: i * 128 + 64 * par + 64] = rsign
    # selO [128, 2*128]: oma broadcast; omar oma rows at 1+h
    selO = np.zeros((128, 256), np.float32)
    for jc in range(2):
        selO[1 + jc * 2, jc * 128: jc * 128 + 64] = 1.0
        selO[2 + jc * 2, jc * 128 + 64: jc * 128 + 128] = 1.0
    # selY [4, 2*128]: sc2 broadcast onto yT head-pair partitions
    selY = np.zeros((4, 256), np.float32)
    for p in range(128):
        selY[p // 64, p] = 1.0
        selY[2 + p // 64, 128 + p] = 1.0
    return selD, selB, selO, selY


_SELD, _SELB, _SELO, _SELY = _build_sels()


def _bf16_resid(a):
    return (a - np.asarray(a.astype(BF16), np.float32)).astype(BF16)


def _shard_inputs(inputs):
    x = np.asarray(inputs["x"], np.float32)
    Wqkv = np.asarray(inputs["Wqkv"], np.float32)
    bqkv = np.asarray(inputs["bqkv"], np.float32)
    Wout = np.asarray(inputs["Wout"], np.float32)
    bout = np.asarray(inputs["bout"], np.float32)
    Wimp = np.asarray(inputs["Wimp"], np.float32)
    bimp = np.asarray(inputs["bimp"], np.float32)
    Walpha = np.asarray(inputs["Walpha"], np.float32)
    balpha = np.asarray(inputs["balpha"], np.float32)
    th = np.asarray(inputs["threshold"], np.float32)

    cmask = np.triu(np.full((128, 128), NEG, np.float32), 1)
    in_maps = []
    for core in range(N_CORES):
        b = core // 4
        hs = (core % 4) * HL
        sl = slice(hs * D, (hs + HL) * D)
        wia = np.zeros((C, 5), np.float32)
        wia[:, 0] = Wimp[0]
        wia[:, 1:5] = Walpha[hs:hs + HL].T
        biad = np.zeros((128, 1), np.float32)
        biad[0, 0] = bimp[0]
        biad[1:5, 0] = balpha[hs:hs + HL]
        m = {
            "xT": np.ascontiguousarray(x[b].T).astype(BF16),
            "xlT": _bf16_resid(np.ascontiguousarray(x[b].T)),
            "wqT": np.ascontiguousarray(Wqkv[sl].T).astype(BF16),
            "wkT": np.ascontiguousarray(
                Wqkv[C + hs * D: C + (hs + HL) * D].T).astype(BF16),
            "wvT": np.ascontiguousarray(
                Wqkv[2 * C + hs * D: 2 * C + (hs + HL) * D].T).astype(BF16),
            "bqd": np.ascontiguousarray(bqkv[sl].reshape(2, 128).T),
            "bkd": np.ascontiguousarray(
                bqkv[C + hs * D: C + (hs + HL) * D].reshape(2, 128).T),
            "bvd": np.ascontiguousarray(
                bqkv[2 * C + hs * D: 2 * C + (hs + HL) * D].reshape(2, 128).T),
            "wia": wia.astype(BF16),
            "biad": biad,
            "woT": np.ascontiguousarray(Wout[:, sl].T).astype(BF16),
            "bout_b": np.ascontiguousarray(np.broadcast_to(bout, (96, C))),
            "th_b": np.full((128, 1), th[0], np.float32),
            "cmask": cmask,
            "selD": _SELD,
            "selB": _SELB,
            "selO": _SELO,
            "selY": _SELY,
        }
        in_maps.append(m)
    return in_maps


def kernel(**inputs):
    nc = _get_nc()
    in_maps = _shard_inputs(inputs)
    trace = os.environ.get("KERNEL_PROFILE", "") == "1"
    res = run_bass_kernel_spmd(
        nc, in_maps, core_ids=list(range(N_CORES)), trace=trace
    )
    KSTATS["exec_time_ns"] = res.exec_time_ns
    return _assemble({c: res.results[c] for c in range(N_CORES)})


def _assemble(results):
    out = np.zeros((B, T, C), np.float32)
    gsz = [384, 256, 256, 128]
    for core in range(N_CORES):
        b, r = core // 4, core % 4
        tb = ob = 0
        for g in range(4):
            s4 = gsz[g] // 4
            out[b, tb + r * s4: tb + (r + 1) * s4, :] = \
                results[core]["out"][ob:ob + s4]
            tb += gsz[g]
            ob += s4
    return out


# revision 36
# speedup vs baseline: 1.0406x; 1.0406x over previous
"""AdaptiveGeometryAttention distributed Bass kernel for 8 trn2 NeuronCores.

Sharding: data-parallel over B (2 groups of 4 cores), head-parallel over H
(4 heads per core). Each core computes its heads' attention and a partial
out-projection [T, C]; per-row-block ReduceScatter(add) chunks over each
4-core group overlap the tail collective with compute.

Design notes (v3):
- bf16 x/weights on host: halves load traffic; PE runs bf16 at 1 cyc/row.
  An extra bf16 residual of x feeds the importance/alpha projection so the
  hard spike threshold (imp > 0.5) cannot flip on rounding.
- Stats phase packs all 8 (side, head) norm rows into partitions 0-7 of one
  tile so each sinh/cosh/rat op covers everything in one 1024-col pass.
- Per-unit attention chain is 2 Act passes (Ln straight from the ni PSUM -
  the clamp is dropped since min ni = 207 over all pairs; Exp with den
  accum) + 3 DVE passes (d2a = L*L; Bt = se_psum - alpha*d2a via STT
  reading PSUM; 128-col diag mask add). Softmax normalization (spike/den)
  is applied to the tiny [d,t] attnv outputs via a PE-broadcast of sc2
  instead of an S-wide pass over the probabilities.
- Tile emits semaphore-recycle fences that wait on the collective clock at
  their program point, so ALL ReduceScatters are EMITTED after the compute
  pipeline while the gpsimd queue is kept empty during phase D: the queue
  head reaches them immediately and each RS still executes early, paced by
  its partial-store DMA semaphore, but no mid-pipeline fence ever waits on
  a collective. RS runs in 4 chunks of [256, C] to amortize the ~12us
  per-collective rendezvous latency; tail loads/stores ride gpsimd.
"""
import os
import sys

for _p in ("/opt/trn_rl_repo",):
    if _p not in sys.path:
        sys.path.append(_p)

import ml_dtypes
import numpy as np
import concourse.bass as bass
import concourse.bacc as bacc
import concourse.mybir as mybir
from concourse import masks
from concourse.alu_op_type import AluOpType
from concourse.tile import TileContext
from concourse.bass_utils import run_bass_kernel_spmd

AF = mybir.ActivationFunctionType
DT = mybir.dt

B, T, C, H, D = 2, 1024, 1024, 16, 64
HL = 4                 # heads per core
JD = HL * D            # 256 local head dims
N_CORES = 8
GROUPS = [[0, 1, 2, 3], [4, 5, 6, 7]]
SQD = 0.125            # 1/sqrt(D)
NEG = -1.0e30
BF16 = ml_dtypes.bfloat16

KSTATS = {}

# The act-table-load placement pass picks the FIRST set containing each
# activation function; strip ln/exp from the single-function sets so the
# combined natural_log_exp_and_others set is chosen (avoids table thrash).
_orig_get_tables = bacc.get_activation_tables


def _patched_get_tables(arch):
    t = _orig_get_tables(arch)
    for nm in ("exp_and_others", "natural_log", "exp_and_friends"):
        if nm in t:
            t[nm] = t[nm] - {AF.Exp, AF.Ln}
    return t


bacc.get_activation_tables = _patched_get_tables


def _f32r(ap):
    return ap.bitcast(DT.float32r)


def build_nc():
    nc = bacc.Bacc("TRN2")

    # ---- I/O ----
    xT_e = nc.dram_tensor("xT", [C, T], DT.bfloat16, kind="ExternalInput")
    xlT_e = nc.dram_tensor("xlT", [C, T], DT.bfloat16, kind="ExternalInput")
    wqT_e = nc.dram_tensor("wqT", [C, JD], DT.bfloat16, kind="ExternalInput")
    wkT_e = nc.dram_tensor("wkT", [C, JD], DT.bfloat16, kind="ExternalInput")
    wvT_e = nc.dram_tensor("wvT", [C, JD], DT.bfloat16, kind="ExternalInput")
    bqd_e = nc.dram_tensor("bqd", [128, 2], DT.float32, kind="ExternalInput")
    bkd_e = nc.dram_tensor("bkd", [128, 2], DT.float32, kind="ExternalInput")
    bvd_e = nc.dram_tensor("bvd", [128, 2], DT.float32, kind="ExternalInput")
    wia_e = nc.dram_tensor("wia", [C, 5], DT.bfloat16, kind="ExternalInput")
    biad_e = nc.dram_tensor("biad", [128, 1], DT.float32, kind="ExternalInput")
    woT_e = nc.dram_tensor("woT", [JD, C], DT.bfloat16, kind="ExternalInput")
    bout_e = nc.dram_tensor("bout_b", [96, C], DT.float32, kind="ExternalInput")
    th_e = nc.dram_tensor("th_b", [128, 1], DT.float32, kind="ExternalInput")
    cmask_e = nc.dram_tensor("cmask", [128, 128], DT.float32, kind="ExternalInput")
    selD_e = nc.dram_tensor("selD", [128, 32], DT.float32r, kind="ExternalInput")
    selB_e = nc.dram_tensor("selB", [128, 512], DT.float32r, kind="ExternalInput")
    selO_e = nc.dram_tensor("selO", [128, 256], DT.float32r, kind="ExternalInput")
    selY_e = nc.dram_tensor("selY", [4, 256], DT.float32r, kind="ExternalInput")
    out_e = nc.dram_tensor("out", [T // 4, C], DT.float32, kind="ExternalOutput")

    # one tensor per row-block chunk: avoids false WAR deps between a
    # chunk's ReduceScatter read and the next chunk's partial store
    GSZ = [256, 256, 256, 256]
    partial_d = [nc.dram_tensor(f"partial_d{i}", [GSZ[i], C], DT.bfloat16)
                 for i in range(4)]
    rs_out_d = [nc.dram_tensor(f"rs_out_d{i}", [GSZ[i] // 4, C], DT.bfloat16)
                for i in range(4)]
    TI2G = {ti: (ti // 2, (ti % 2) * 128) for ti in range(8)}

    with TileContext(nc) as tc:
        with (
            tc.tile_pool(name="const", bufs=1) as cpool,
            tc.tile_pool(name="mainp", bufs=1) as mp,
        ):
            # ---- constants (small, on the gpsimd/vector queues) ----
            idf = cpool.tile([128, 128], DT.float32, tag="idf")
            masks.make_identity(nc, idf[:])
            cmask = cpool.tile([128, 128], DT.float32, tag="cmask")
            nc.gpsimd.dma_start(out=cmask[:], in_=cmask_e[:])
            selD = cpool.tile([128, 32], DT.float32r, tag="selD")
            nc.gpsimd.dma_start(out=selD[:], in_=selD_e[:])
            selB = cpool.tile([128, 512], DT.float32r, tag="selB")
            nc.gpsimd.dma_start(out=selB[:], in_=selB_e[:])
            selO = cpool.tile([128, 256], DT.float32r, tag="selO")
            nc.gpsimd.dma_start(out=selO[:], in_=selO_e[:])
            selY = cpool.tile([4, 256], DT.float32r, tag="selY")
            nc.gpsimd.dma_start(out=selY[:], in_=selY_e[:])
            th_b = cpool.tile([128, 1], DT.float32, tag="thb")
            nc.gpsimd.dma_start(out=th_b[:], in_=th_e[:])
            biad = cpool.tile([128, 1], DT.float32, tag="biad")
            nc.gpsimd.dma_start(out=biad[:], in_=biad_e[:])
            bout_b = cpool.tile([96, C], DT.float32, tag="boutb")
            nc.gpsimd.dma_start(out=bout_b[:], in_=bout_e[:])

            # ---- persistent main tiles ----
            qb2T = [mp.tile([128, T], DT.bfloat16, tag=f"qb2T{j}", name=f"qb2T{j}") for j in range(2)]
            kbT = [mp.tile([128, T], DT.bfloat16, tag=f"kbT{j}", name=f"kbT{j}") for j in range(2)]
            qhT = [mp.tile([128, T], DT.float32r, tag=f"qhT{j}", name=f"qhT{j}") for j in range(2)]
            khT = [mp.tile([128, T], DT.float32r, tag=f"khT{j}", name=f"khT{j}") for j in range(2)]
            vbf = mp.tile([128, 8 * JD], DT.bfloat16, tag="vbf")
            wobf = mp.tile([128, 2 * C], DT.bfloat16, tag="wobf")
            for cc in range(2):
                nc.gpsimd.dma_start(
                    out=wobf[:, cc * C:(cc + 1) * C],
                    in_=woT_e[cc * 128:(cc + 1) * 128, :],
                )
            sat = mp.tile([128, 64], DT.float32, tag="sat")
            nsat = mp.tile([128, 64], DT.float32, tag="nsat")
            spike = mp.tile([128, 8], DT.float32, tag="spike")

            with tc.tile_pool(name="wB", bufs=1) as wb, \
                 tc.tile_pool(name="psB", bufs=3, space="PSUM") as psB:
                # ---- load x/weights; wv first so the v projection can
                # start as soon as the matching xT chunks land ----
                # even wv/x chunks on sync, everything else on the scalar
                # queue: Sync's phase-D job is ONLY the pT transposes, so its
                # semaphore epochs never wait on slow consumers; Act does no
                # DMAs after the load phase for the same reason.
                xT = wb.tile([128, 8 * T], DT.bfloat16, tag="xT")
                wv = wb.tile([128, 8 * JD], DT.bfloat16, tag="wv")
                wq = wb.tile([128, 8 * JD], DT.bfloat16, tag="wq")
                wk = wb.tile([128, 8 * JD], DT.bfloat16, tag="wk")
                wiar = wb.tile([128, 40], DT.bfloat16, tag="wiar")
                xlT = wb.tile([128, 8 * T], DT.bfloat16, tag="xlT")
                for kc in range(8):
                    nc.sync.dma_start(
                        out=wq[:, kc * JD:(kc + 1) * JD],
                        in_=wqT_e[kc * 128:(kc + 1) * 128, :],
                    )
                    nc.scalar.dma_start(
                        out=wk[:, kc * JD:(kc + 1) * JD],
                        in_=wkT_e[kc * 128:(kc + 1) * 128, :],
                    )
                for kc in range(8):
                    q = nc.sync if kc % 2 == 0 else nc.scalar
                    q.dma_start(
                        out=xT[:, kc * T:(kc + 1) * T],
                        in_=xT_e[kc * 128:(kc + 1) * 128, :],
                    )
                for kc in range(8):
                    q = nc.sync if kc % 2 == 0 else nc.scalar
                    q.dma_start(
                        out=wv[:, kc * JD:(kc + 1) * JD],
                        in_=wvT_e[kc * 128:(kc + 1) * 128, :],
                    )
                # xl: bf16 residual of x, for the threshold-sensitive
                # importance/alpha projection (imp > th is a hard compare)
                for w_t, w_e, jw, q in ((wiar, wia_e, 5, nc.scalar),
                                        (xlT, xlT_e, T, nc.gpsimd)):
                    for kc in range(8):
                        q.dma_start(
                            out=w_t[:, kc * jw:(kc + 1) * jw],
                            in_=w_e[kc * 128:(kc + 1) * 128, :],
                        )
                bqd = wb.tile([128, 2], DT.float32, tag="bqd")
                bkd = wb.tile([128, 2], DT.float32, tag="bkd")
                bvd = wb.tile([128, 2], DT.float32, tag="bvd")
                nc.scalar.dma_start(out=bqd[:], in_=bqd_e[:])
                nc.scalar.dma_start(out=bkd[:], in_=bkd_e[:])
                nc.scalar.dma_start(out=bvd[:], in_=bvd_e[:])

                # ---- phase B: projections, weight-stationary [d, t] ----
                qT = [wb.tile([128, T], DT.float32, tag=f"qT{j}", name=f"qT{j}")
                      for j in range(2)]
                kT = [wb.tile([128, T], DT.float32, tag=f"kT{j}", name=f"kT{j}")
                      for j in range(2)]
                for dsts, w_t, b_t in ((qT, wq, bqd), (kT, wk, bkd)):
                    for jc in range(2):
                        pw = psB.tile([128, 1024], DT.float32, tag="ps")
                        for tc2 in range(2):
                            for kc in range(8):
                                nc.tensor.matmul(
                                    pw[:, tc2 * 512:(tc2 + 1) * 512],
                                    w_t[:, kc * JD + jc * 128:
                                        kc * JD + (jc + 1) * 128],
                                    xT[:, kc * T + tc2 * 512:
                                       kc * T + tc2 * 512 + 512],
                                    start=(kc == 0), stop=(kc == 7),
                                )
                        nc.vector.tensor_scalar(
                            dsts[jc][:], pw[:], b_t[:, jc:jc + 1], None,
                            AluOpType.add
                        )

                for jc in range(2):
                    pw = psB.tile([128, 1024], DT.float32, tag="ps")
                    for tc2 in range(2):
                        for kc in range(8):
                            nc.tensor.matmul(
                                pw[:, tc2 * 512:(tc2 + 1) * 512],
                                wv[:, kc * JD + jc * 128:
                                   kc * JD + (jc + 1) * 128],
                                xT[:, kc * T + tc2 * 512:
                                   kc * T + tc2 * 512 + 512],
                                start=(kc == 0), stop=(kc == 7),
                            )
                    vtmp = wb.tile([128, T], DT.bfloat16, tag=f"vtmp{jc}",
                                   name=f"vtmp{jc}")
                    nc.vector.tensor_scalar(
                        vtmp[:], pw[:], bvd[:, jc:jc + 1], None, AluOpType.add
                    )
                    # [d,t] -> row layout [s, d] per 128-block
                    nc.scalar.dma_start_transpose(
                        vbf[:].rearrange("p (s c) -> p s c", c=JD)
                        [:, :, jc * 128:(jc + 1) * 128],
                        vtmp[:],
                    )

                # ia projection: rows 0-4 = imp, alpha h0-3
                iaS = wb.tile([128, T], DT.float32, tag="iaS")
                nc.gpsimd.memset(iaS[:], 0.0)
                pia = psB.tile([128, 1024], DT.float32, tag="ps", name="pia")
                for tc2 in range(2):
                    for kc in range(16):
                        xsrc = xT if kc < 8 else xlT
                        kc8 = kc % 8
                        nc.tensor.matmul(
                            pia[0:5, tc2 * 512:(tc2 + 1) * 512],
                            wiar[:, kc8 * 5:(kc8 + 1) * 5],
                            xsrc[:, kc8 * T + tc2 * 512:
                                 kc8 * T + tc2 * 512 + 512],
                            start=(kc == 0), stop=(kc == 15),
                        )
                nc.scalar.activation(iaS[0:5, :], pia[0:5, :], AF.Sigmoid,
                                     bias=biad[0:5, :])
                # importance[t=0] := 0 before threshold compare
                nc.vector.memset(iaS[0:1, 0:1], 0.0)
                # oma rows 32-36: 1 - (imp, alpha) (row 32 unused)
                nc.vector.tensor_scalar(
                    iaS[32:37, :], iaS[0:5, :], -1.0, 1.0,
                    AluOpType.mult, AluOpType.add,
                )

                # ---- phase C: per-(side, head) norm stats in rows 0-7 ----
                # S8/sq/omar are f32r: the BIR verifier requires f32r matmul
                # operands to be emitted as f32r by their producers (DVE ops
                # round on write; memset goes through an f32 view).
                S8 = wb.tile([128, T], DT.float32r, tag="S8")
                nc.gpsimd.memset(S8[:].bitcast(DT.float32), 0.0)
                omar = wb.tile([128, T], DT.float32r, tag="omar")
                nc.vector.tensor_copy(omar[0:8, :], iaS[32:40, :])
                t1 = wb.tile([128, T], DT.float32, tag="t1")
                n8 = wb.tile([128, T], DT.float32, tag="n8")
                ep = wb.tile([128, T], DT.float32, tag="ep")
                em = wb.tile([128, T], DT.float32, tag="em")
                sq = [wb.tile([128, T], DT.float32r, tag=f"sq{i}",
                              name=f"sq{i}") for i in range(2)]
                pnsq = psB.tile([128, 1024], DT.float32, tag="ps", name="pnsq")
                srcs4 = [qT[0], qT[1], kT[0], kT[1]]
                for i, src in enumerate(srcs4):
                    sqt = sq[i % 2]
                    nc.vector.tensor_mul(sqt[:], src[:], src[:])
                    for tc2 in range(2):
                        nc.tensor.matmul(
                            pnsq[0:8, tc2 * 512:(tc2 + 1) * 512],
                            selD[:, i * 8:(i + 1) * 8],
                            sqt[:, tc2 * 512:(tc2 + 1) * 512],
                            start=(i == 0), stop=(i == 3),
                        )
                # n = exp(0.5 ln n^2); sinh/cosh from exp(+-n); 1/n as
                # exp(-0.5 ln n^2) (DVE reciprocal costs ~6.5us here)
                nc.scalar.activation(t1[0:8, :], pnsq[0:8, :], AF.Ln)
                nc.scalar.activation(n8[0:8, :], t1[0:8, :], AF.Exp, scale=0.5)
                nc.scalar.activation(ep[0:8, :], n8[0:8, :], AF.Exp)
                nc.scalar.activation(em[0:8, :], n8[0:8, :], AF.Exp, scale=-1.0)
                nr8 = wb.tile([128, T], DT.float32, tag="nr8")
                nc.scalar.activation(nr8[0:8, :], t1[0:8, :], AF.Exp,
                                     scale=-0.5)
                nc.vector.tensor_add(S8[32:40, :], ep[0:8, :], em[0:8, :])
                nc.vector.tensor_sub(t1[0:8, :], ep[0:8, :], em[0:8, :])
                nc.vector.tensor_mul(S8[0:8, :], t1[0:8, :], nr8[0:8, :])

                # ---- phase C2: modified q/k via broadcast matmuls ----
                # qb2T needs the ORIGINAL q rows; kbT is a cast of kT; then
                # overwrite q/k d0 rows with 1.0 so the rat/cosh broadcast
                # lands cosh directly in qhT/khT rows 0 and 64.
                for jc in range(2):
                    pbo = psB.tile([128, 1024], DT.float32, tag="ps",
                                   name=f"pbo{jc}")
                    for tc2 in range(2):
                        sl = slice(tc2 * 512, (tc2 + 1) * 512)
                        nc.tensor.matmul(
                            pbo[:, sl], selO[0:8, jc * 128:(jc + 1) * 128],
                            omar[0:8, sl],
                            start=True, stop=True,
                        )
                    nc.vector.tensor_mul(qb2T[jc][:], qT[jc][:], pbo[:])
                    nc.scalar.copy(kbT[jc][:], kT[jc][:])
                for jc in range(2):
                    for rt in (qT[jc], kT[jc]):
                        nc.gpsimd.memset(rt[0:1, :], 1.0)
                        nc.gpsimd.memset(rt[64:65, :], 1.0)
                for side, (srcs, dsts) in enumerate(((qT, qhT), (kT, khT))):
                    for jc in range(2):
                        pbb = psB.tile([128, 1024], DT.float32, tag="ps",
                                       name=f"pbb{side}{jc}")
                        for tc2 in range(2):
                            sl = slice(tc2 * 512, (tc2 + 1) * 512)
                            nc.tensor.matmul(
                                pbb[:, sl],
                                selB[0:48, (side * 2 + jc) * 128:
                                     (side * 2 + jc + 1) * 128],
                                S8[0:48, sl],
                                start=True, stop=True,
                            )
                        nc.vector.tensor_mul(dsts[jc][:], srcs[jc][:], pbb[:])

                # transposed per-t stats: sat cols t8*8 + (imp, alpha h0-3)
                pst = psB.tile([128, 1024], DT.float32, tag="ps", name="pst")
                for t8 in range(8):
                    nc.tensor.transpose(
                        pst[:, t8 * 128:(t8 + 1) * 128],
                        iaS[:, t8 * 128:(t8 + 1) * 128],
                        idf[:],
                    )
                nc.vector.tensor_copy(
                    sat[:].rearrange("p (b c) -> p b c", c=8)[:, :, 0:5],
                    pst[:].rearrange("p (b c) -> p b c", c=128)[:, :, 0:5],
                )
                nc.vector.tensor_scalar(nsat[:], sat[:], -1.0, None,
                                        AluOpType.mult)
                nc.vector.tensor_scalar(spike[:], sat[:, 0::8], th_b[:], None,
                                        AluOpType.is_gt)

            # ---- phase D: attention, software-pipelined ----
            # per unit (ti, h): PE se/ni; Act Ln(ni psum); DVE d2a=L*L,
            # Bt=se-alpha*d2a (STT from psum); Pool diag mask; Act Exp+accum.
            # per ti: DMA transposes -> pT; PE attnv + sc2 broadcast; DVE yT
            # scale; PE out-proj; Pool out copy; Act partial store; RS.
            with tc.tile_pool(name="pipe", bufs=1) as pp, \
                 tc.tile_pool(name="pipes", bufs=4) as sp, \
                 tc.tile_pool(name="pipeo", bufs=4) as po, \
                 tc.tile_pool(name="psS", bufs=2, space="PSUM") as psS, \
                 tc.tile_pool(name="psN", bufs=2, space="PSUM") as psN:
                UNITS = [(ti, h) for ti in range(8) for h in range(HL)]
                NU = len(UNITS)
                st = {}
                den_t = {}
                pT_t = {}
                sc_t = {}

                def stage0(u):
                    ti, h = UNITS[u]
                    S = (ti + 1) * 128
                    jc, hh = h // 2, h % 2
                    se = psS.tile([128, 1024], DT.float32, tag="se", name=f"se{u}")
                    ni = psN.tile([128, 1024], DT.float32, tag="ni", name=f"ni{u}")
                    for c0 in range(0, S, 512):
                        n_sc = min(512, S - c0)
                        nc.tensor.matmul(
                            se[:, c0:c0 + n_sc],
                            qb2T[jc][hh * 64:(hh + 1) * 64, ti * 128:(ti + 1) * 128],
                            kbT[jc][hh * 64:(hh + 1) * 64, c0:c0 + n_sc],
                            start=True, stop=True,
                        )
                    for c0 in range(0, S, 512):
                        n_sc = min(512, S - c0)
                        nc.tensor.matmul(
                            ni[:, c0:c0 + n_sc],
                            qhT[jc][hh * 64:(hh + 1) * 64,
                                    ti * 128:(ti + 1) * 128],
                            khT[jc][hh * 64:(hh + 1) * 64, c0:c0 + n_sc],
                            start=True, stop=True,
                        )
                    st[u] = (se, ni)

                def stage12(u):
                    ti, h = UNITS[u]
                    S = (ti + 1) * 128
                    se, ni = st.pop(u)
                    L = pp.tile([128, 1024], DT.float32, tag="L", bufs=3,
                                name=f"L{u}")
                    nc.scalar.activation(L[:, :S], ni[:, :S], AF.Ln, scale=2.0)
                    d2a = pp.tile([128, 1024], DT.float32, tag="d2a", bufs=3,
                                  name=f"d2a{u}")
                    nc.vector.tensor_mul(d2a[:, :S], L[:, :S], L[:, :S])
                    Bt = pp.tile([128, 1024], DT.float32, tag="Bt", bufs=4,
                                 name=f"Bt{u}")
                    sac = ti * 8 + 1 + h
                    nc.vector.scalar_tensor_tensor(
                        Bt[:, :S], d2a[:, :S], nsat[:, sac:sac + 1], se[:, :S],
                        AluOpType.mult, AluOpType.add,
                    )
                    nc.vector.tensor_add(
                        Bt[:, ti * 128:S], Bt[:, ti * 128:S], cmask[:]
                    )
                    st[u] = Bt

                pb4_t = {}
                psy_t = {}

                def stage3(u):
                    ti, h = UNITS[u]
                    S = (ti + 1) * 128
                    Bt = st.pop(u)
                    if h == 0:
                        den_t[ti] = sp.tile([128, 4], DT.float32, tag="den",
                                            name=f"den{ti}")
                        pb4_t[ti] = pp.tile([128, 4 * 1024], DT.bfloat16,
                                            tag="pb4", bufs=2, name=f"pb4{ti}")
                    den = den_t[ti]
                    pb4 = pb4_t[ti]
                    nc.scalar.activation(pb4[:, h * S:(h + 1) * S],
                                         Bt[:, :S],
                                         AF.Exp,
                                         scale=SQD, accum_out=den[:, h:h + 1])
                    if ti == 7:
                        # last ti: per-head transpose + attnv right after each
                        # Exp so the drain chain is one head, not four. psy7
                        # lives in the se ring (nothing allocs there later).
                        if h == 0:
                            psy_t[ti] = psS.tile([128, 1024], DT.float32,
                                                 tag="se", name="psy7")
                            pT_t[ti] = pp.tile([128, 4 * 1024], DT.bfloat16,
                                               tag="pT4", bufs=3, name="pT47")
                        pT47 = pT_t[ti]
                        nc.sync.dma_start_transpose(
                            pT47[:, h * S:(h + 1) * S].rearrange(
                                "p (b c) -> p b c", c=128),
                            pb4[:, h * S:(h + 1) * S],
                        )
                        jc2, hh2 = h // 2, h % 2
                        for sj in range(ti + 1):
                            nc.tensor.matmul(
                                psy_t[ti][hh2 * 64:(hh2 + 1) * 64,
                                          jc2 * 128:(jc2 + 1) * 128],
                                vbf[:, sj * JD + h * D: sj * JD + (h + 1) * D],
                                pT47[:, h * S + sj * 128:
                                     h * S + (sj + 1) * 128],
                                start=(sj == 0), stop=(sj == ti),
                                tile_position=(0, hh2 * 64),
                            )
                    if h == HL - 1:
                        den4 = den_t.pop(ti)
                        rec4 = sp.tile([128, 4], DT.float32, tag="rec4")
                        nc.vector.reciprocal(rec4[:], den4[:])
                        sc24 = sp.tile([128, 4], DT.float32, tag="sc24",
                                       name=f"sc24{ti}")
                        nc.vector.tensor_scalar(
                            sc24[:], rec4[:], spike[:, ti:ti + 1], None,
                            AluOpType.mult,
                        )
                        sc_t[ti] = sc24
                        pb4 = pb4_t.pop(ti)
                        if ti != 7:
                            pT4 = pp.tile([128, 4 * 1024], DT.bfloat16,
                                          tag="pT4", bufs=3, name=f"pT4{ti}")
                            # heads are packed S-contiguously, so one DMA
                            # transposes exactly the needed 4*S columns
                            nc.sync.dma_start_transpose(
                                pT4[:, :4 * S].rearrange("p (b c) -> p b c",
                                                         c=128),
                                pb4[:, :4 * S],
                            )
                            pT_t[ti] = pT4

                def stage4(u):
                    ti, h = UNITS[u]
                    if h != HL - 1:
                        return
                    pT4 = pT_t.pop(ti)
                    S4 = (ti + 1) * 128
                    sc24 = sc_t.pop(ti)
                    if ti == 7:
                        psy = psy_t.pop(ti)
                    else:
                        psy = psN.tile([128, 1024], DT.float32, tag="ni",
                                       name=f"psy{ti}")
                    # sc2 [t,h] -> [h,t] -> per-head-pair broadcast [d, t]
                    nc.tensor.transpose(psy[0:4, 512:640], sc24[:], idf[:])
                    scT = sp.tile([4, 128], DT.float32r, tag="scT")
                    nc.vector.tensor_copy(scT[:], psy[0:4, 512:640])
                    for half in range(2):
                        nc.tensor.matmul(
                            psy[:, 768 + half * 128:768 + (half + 1) * 128],
                            selY[:, half * 128:(half + 1) * 128],
                            scT[:],
                            start=True, stop=True,
                        )
                    for h2 in (range(HL) if ti != 7 else ()):
                        jc2, hh2 = h2 // 2, h2 % 2
                        for sj in range(ti + 1):
                            nc.tensor.matmul(
                                psy[hh2 * 64:(hh2 + 1) * 64,
                                    jc2 * 128:(jc2 + 1) * 128],
                                vbf[:, sj * JD + h2 * D: sj * JD + (h2 + 1) * D],
                                pT4[:, h2 * S4 + sj * 128:
                                    h2 * S4 + (sj + 1) * 128],
                                start=(sj == 0), stop=(sj == ti),
                                tile_position=(0, hh2 * 64),
                            )
                    sc2b = sp.tile([128, 256], DT.float32, tag="sc2b")
                    nc.vector.tensor_copy(sc2b[:], psy[:, 768:1024])
                    yTm = sp.tile([128, 256], DT.bfloat16, tag="yTm")
                    nc.vector.tensor_mul(yTm[:], psy[:, 0:256], sc2b[:])
                    yT0, yT1 = yTm[:, 0:128], yTm[:, 128:256]
                    pso = psN.tile([128, 1024], DT.float32, tag="ni",
                                   name=f"pso{ti}")
                    for oc in range(2):
                        for cc, yT_t in ((0, yT0), (1, yT1)):
                            nc.tensor.matmul(
                                pso[:, oc * 512:(oc + 1) * 512],
                                yT_t,
                                wobf[:, cc * C + oc * 512: cc * C + oc * 512 + 512],
                                start=(cc == 0), stop=(cc == 1),
                            )
                    out_sb = po.tile([128, 1024], DT.bfloat16, tag="outsb")
                    nc.vector.tensor_copy(out_sb[:], pso[:])
                    # partial store on the Pool queue (Sync carries only the
                    # pT transposes, Act only compute). The RS for this chunk
                    # is issued 2 tis LATER: semaphore-recycle fences wait on
                    # the CC clock at their program point, so late issuance
                    # keeps mid-pipeline fences free of collective waits.
                    g_, off_ = TI2G[ti]
                    nc.sync.dma_start(
                        out=partial_d[g_][off_:off_ + 128, :],
                        in_=out_sb[:])

                def _issue_rs(g):
                    nc.gpsimd.collective_compute(
                        "ReduceScatter", mybir.AluOpType.add,
                        replica_groups=GROUPS,
                        ins=[partial_d[g][:]],
                        outs=[rs_out_d[g][:]],
                    )

                for step in range(NU + 6):
                    if step < NU:
                        stage0(step)
                    if 1 <= step < NU + 1:
                        stage12(step - 1)
                    if 2 <= step < NU + 2:
                        stage3(step - 2)
                    if 6 <= step < NU + 6:
                        stage4(step - 6)

                for g in range(4):
                    _issue_rs(g)

                # ---- tail: bias add + output store per RS chunk, on the
                # Pool queue so Sync never waits on collective sems ----
                ob = 0
                for g in range(4):
                    sz4 = GSZ[g] // 4
                    finb = po.tile([96, 1024], DT.bfloat16, tag="finb")
                    nc.gpsimd.dma_start(out=finb[0:sz4, :], in_=rs_out_d[g][:])
                    fin = po.tile([96, 1024], DT.float32, tag="fin")
                    nc.vector.tensor_add(fin[0:sz4, :], finb[0:sz4, :],
                                         bout_b[0:sz4, :])
                    nc.gpsimd.dma_start(
                        out=out_e[ob:ob + sz4, :], in_=fin[0:sz4, :]
                    )
                    ob += sz4
    nc.finalize()
    return nc


_NC = None


def _get_nc():
    global _NC
    if _NC is None:
        _NC = build_nc()
    return _NC


def _build_sels():
    # selD [128, 4*8]: per (side, jc) block, col 2h+par sums the d rows of
    # head parity par (excluding d0) for the norm^2 reduction
    selD = np.zeros((128, 32), np.float32)
    for i in range(4):            # (q,0),(q,1),(k,0),(k,1)
        side, jc = i // 2, i % 2
        r0 = side * 4 + jc * 2
        selD[1:64, i * 8 + r0] = 1.0
        selD[65:128, i * 8 + r0 + 1] = 1.0
    # selB [128, 4*128]: rat/cosh broadcast. S8 rows: rat at side*4+h,
    # 2cosh at 32+side*4+h. q side +0.5 rat, k side -0.5 (metric sign).
    selB = np.zeros((128, 512), np.float32)
    for i in range(4):
        side, jc = i // 2, i % 2
        rsign = 0.5 if side == 0 else -0.5
        for par in range(2):
            hrow = side * 4 + jc * 2 + par
            selB[32 + hrow, i * 128 + 64 * par] = 0.5
            selB[hrow, i * 128 + 64 * par + 1: i * 128 + 64 * par + 64] = rsign
    # selO [128, 2*128]: oma broadcast; omar oma rows at 1+h
    selO = np.zeros((128, 256), np.float32)
    for jc in range(2):
        selO[1 + jc * 2, jc * 128: jc * 128 + 64] = 1.0
        selO[2 + jc * 2, jc * 128 + 64: jc * 128 + 128] = 1.0
    # selY [4, 2*128]: sc2 broadcast onto yT head-pair partitions
    selY = np.zeros((4, 256), np.float32)
    for p in range(128):
        selY[p // 64, p] = 1.0
        selY[2 + p // 64, 128 + p] = 1.0
    return selD, selB, selO, selY


_SELD, _SELB, _SELO, _SELY = _build_sels()


def _bf16_resid(a):
    return (a - np.asarray(a.astype(BF16), np.float32)).astype(BF16)


def _shard_inputs(inputs):
    x = np.asarray(inputs["x"], np.float32)
    Wqkv = np.asarray(inputs["Wqkv"], np.float32)
    bqkv = np.asarray(inputs["bqkv"], np.float32)
    Wout = np.asarray(inputs["Wout"], np.float32)
    bout = np.asarray(inputs["bout"], np.float32)
    Wimp = np.asarray(inputs["Wimp"], np.float32)
    bimp = np.asarray(inputs["bimp"], np.float32)
    Walpha = np.asarray(inputs["Walpha"], np.float32)
    balpha = np.asarray(inputs["balpha"], np.float32)
    th = np.asarray(inputs["threshold"], np.float32)

    cmask = np.triu(np.full((128, 128), NEG, np.float32), 1)
    in_maps = []
    for core in range(N_CORES):
        b = core // 4
        hs = (core % 4) * HL
        sl = slice(hs * D, (hs + HL) * D)
        wia = np.zeros((C, 5), np.float32)
        wia[:, 0] = Wimp[0]
        wia[:, 1:5] = Walpha[hs:hs + HL].T
        biad = np.zeros((128, 1), np.float32)
        biad[0, 0] = bimp[0]
        biad[1:5, 0] = balpha[hs:hs + HL]
        m = {
            "xT": np.ascontiguousarray(x[b].T).astype(BF16),
            "xlT": _bf16_resid(np.ascontiguousarray(x[b].T)),
            "wqT": np.ascontiguousarray(Wqkv[sl].T).astype(BF16),
            "wkT": np.ascontiguousarray(
                Wqkv[C + hs * D: C + (hs + HL) * D].T).astype(BF16),
            "wvT": np.ascontiguousarray(
                Wqkv[2 * C + hs * D: 2 * C + (hs + HL) * D].T).astype(BF16),
            "bqd": np.ascontiguousarray(bqkv[sl].reshape(2, 128).T),
            "bkd": np.ascontiguousarray(
                bqkv[C + hs * D: C + (hs + HL) * D].reshape(2, 128).T),
            "bvd": np.ascontiguousarray(
                bqkv[2 * C + hs * D: 2 * C + (hs + HL) * D].reshape(2, 128).T),
            "wia": wia.astype(BF16),
            "biad": biad,
            "woT": np.ascontiguousarray(Wout[:, sl].T).astype(BF16),
            "bout_b": np.ascontiguousarray(np.broadcast_to(bout, (96, C))),
            "th_b": np.full((128, 1), th[0], np.float32),
            "cmask": cmask,
            "selD": _SELD,
            "selB": _SELB,
            "selO": _SELO,
            "selY": _SELY,
        }
        in_maps.append(m)
    return in_maps


def kernel(**inputs):
    nc = _get_nc()
    in_maps = _shard_inputs(inputs)
    trace = os.environ.get("KERNEL_PROFILE", "") == "1"
    res = run_bass_kernel_spmd(
        nc, in_maps, core_ids=list(range(N_CORES)), trace=trace
    )
    KSTATS["exec_time_ns"] = res.exec_time_ns
    return _assemble({c: res.results[c] for c in range(N_CORES)})


def _assemble(results):
    out = np.zeros((B, T, C), np.float32)
    gsz = [256, 256, 256, 256]
    for core in range(N_CORES):
        b, r = core // 4, core % 4
        tb = ob = 0
        for g in range(4):
            s4 = gsz[g] // 4
            out[b, tb + r * s4: tb + (r + 1) * s4, :] = \
                results[core]["out"][ob:ob + s4]
            tb += gsz[g]
            ob += s4
    return out
